# revision 30
# baseline (speedup 1.0000x reference)
"""Causal cross-attention Trainium2 kernel.

Sharding: 8 cores = 2 (batch) x 4 (head-groups of 4 heads, tensor parallel).
Each core computes its batch's attention for its 4 heads plus the partial
(row-parallel) output projection; the host sums head-group partials + bo.

Device layout is fully transposed (Q^T/K^T = [d, seq], scores = S^T[k, q],
output Y^T = [D, SQ]) so that:
  - the softmax denominator comes free as a fused ones-column in the PV matmul
  - Q/K biases are per-partition adds
  - the out-projection streams OT as the moving operand (few weight loads)
No max-subtraction in softmax: scores are ~N(0,1) (scale 1/sqrt(64) folded
into Wq), so raw exp is safe in fp32.

Schedule:
  - startup: full Q/K projections (both 128-dim halves) stream against the
    input DMA; Q/K bias adds ride the Scalar engine; exp ACT table preloaded.
  - phase A (heads 0,1): attention with V-projection matmuls interleaved
    into PE bubbles; per-pair softmax divide trails each q-block.
  - phase B (heads 2,3): attention + divide + out-projection, with
    PSUM->SBUF casts split between Vector and Scalar engines.
Attention processes head pairs concurrently: the two K=64 score matmuls are
row-packed into disjoint PE row groups (partitions 0-63 / 64-127), their
outputs share one 2-bank PSUM tile, and a single strided ACT exp covers both.
"""

import sys

if "/opt/trn_rl_repo" not in sys.path:
    sys.path.insert(0, "/opt/trn_rl_repo")

import numpy as np

import concourse.mybir as mybir
import concourse.tile as tile
from concourse import bacc
from concourse.bass_utils import run_bass_kernel_spmd

# problem shapes (hardcoded)
B = 2
SQ = 2048
SKV = 2048
D = 1024
H = 16
DH = 64
N_CORES = 8
HG = 4  # head groups
H_PER = H // HG  # 4 heads per core
DG = H_PER * DH  # 256 dims per core

F16 = mybir.dt.float16
F32 = mybir.dt.float32

QB = 512  # q block (free dim per matmul)
KT = 128  # kv tile (partition dim)
NQB = SQ // QB  # 4
NKT = SKV // KT  # 16
NCT = D // 128  # 8 contraction tiles for projections
NDT = DG // 128  # 2 partition tiles for the 256 head dims

_CACHE = {}


def _build():
    nc = bacc.Bacc("TRN2", target_bir_lowering=False, debug=False)

    qT_ext = nc.dram_tensor("qT", [D, SQ], F16, kind="ExternalInput")
    kvT_ext = nc.dram_tensor("kvT", [D, SKV], F16, kind="ExternalInput")
    wqT_ext = nc.dram_tensor("wqT", [D, DG], F16, kind="ExternalInput")
    wkT_ext = nc.dram_tensor("wkT", [D, DG], F16, kind="ExternalInput")
    wvT_ext = nc.dram_tensor("wvT", [D, DG], F16, kind="ExternalInput")
    woT_ext = nc.dram_tensor("woT", [DG, D], F16, kind="ExternalInput")
    bqk_ext = nc.dram_tensor("bqk", [128, 2 * NDT], F32, kind="ExternalInput")
    bv_ext = nc.dram_tensor("bv", [1, DG], F16, kind="ExternalInput")
    mask_ext = nc.dram_tensor("mask", [128, 128], F16, kind="ExternalInput")
    yT_ext = nc.dram_tensor("yT", [D, SQ], F16, kind="ExternalOutput")
    dbg = {}
    if _CACHE.get("debug"):
        dbg["v"] = nc.dram_tensor(
            "dbg_v", [128, NKT * H_PER * (DH + 1)], F16, kind="ExternalOutput"
        )
        dbg["qt"] = nc.dram_tensor(
            "dbg_qt", [128, NDT * SQ], F16, kind="ExternalOutput"
        )
        dbg["kt"] = nc.dram_tensor(
            "dbg_kt", [128, NDT * SKV], F16, kind="ExternalOutput"
        )
        dbg["ot"] = nc.dram_tensor(
            "dbg_ot", [128, NDT * SQ], F16, kind="ExternalOutput"
        )
        dbg["r"] = nc.dram_tensor(
            "dbg_r", [33, 8 * QB], F16, kind="ExternalOutput"
        )
        dbg["osb"] = nc.dram_tensor(
            "dbg_osb", [DH + 1, 16 * QB], F16, kind="ExternalOutput"
        )

    with tile.TileContext(nc) as tc:
        with (
            tc.tile_pool(name="res", bufs=1) as res,
            tc.tile_pool(name="pp", bufs=6) as pp,
            tc.tile_pool(name="yp", bufs=6) as yp,
            tc.tile_pool(name="rp", bufs=3) as rp,
            tc.tile_pool(name="op", bufs=5) as op,
        ):
            # ---- inputs on two DMA rings (sync + scalar), ordered by ----
            # consumption time: wq first, q tiles alternate rings, then
            # K-side, then V/O weights. Keeps the PE fed from ~2us on.
            qT_view = qT_ext.rearrange("(kt p) q -> kt p q", p=128)
            kvT_view = kvT_ext.rearrange("(kt p) q -> kt p q", p=128)
            qT_kt = [
                res.tile([128, SQ], F16, tag=f"qT{kt}", name=f"qT{kt}")
                for kt in range(NCT)
            ]
            kvT_kt = [
                res.tile([128, SKV], F16, tag=f"kvT{kt}", name=f"kvT{kt}")
                for kt in range(NCT)
            ]
            wq_s = res.tile([128, NCT, DG], F16)
            wk_s = res.tile([128, NCT, DG], F16)
            wv_s = res.tile([128, NCT, DG], F16)
            wo_s = res.tile([128, NDT, D], F16)
            bqk_s = res.tile([128, 2 * NDT], F32)
            bv_s = res.tile([1, DG], F16)
            mask_s = res.tile([128, 128], F16)

            # wq arrives in per-kt chunks so Q-proj kt=0 starts after the
            # first 64KB instead of the whole 512KB rearrange
            wq_view = wqT_ext.rearrange("(kt p) d -> kt p d", p=128)
            for kt in range(NCT):
                nc.scalar.dma_start(out=wq_s[:, kt, :], in_=wq_view[kt])
            for kt in range(0, NCT, 2):
                nc.sync.dma_start(out=qT_kt[kt], in_=qT_view[kt])
            for kt in range(1, NCT, 2):
                nc.scalar.dma_start(out=qT_kt[kt], in_=qT_view[kt])
            nc.scalar.dma_start(out=bqk_s, in_=bqk_ext[:, :])
            nc.scalar.dma_start(
                out=wk_s, in_=wkT_ext.rearrange("(kt p) d -> p kt d", p=128)
            )
            for kt in range(0, NCT, 2):
                nc.sync.dma_start(out=kvT_kt[kt], in_=kvT_view[kt])
            for kt in range(1, NCT, 2):
                nc.scalar.dma_start(out=kvT_kt[kt], in_=kvT_view[kt])
            nc.scalar.dma_start(
                out=wv_s, in_=wvT_ext.rearrange("(kt p) d -> p kt d", p=128)
            )
            nc.scalar.dma_start(out=bv_s, in_=bv_ext[:, :])
            nc.scalar.dma_start(out=mask_s, in_=mask_ext[:, :])
            # preload the exp ACT table before the out-proj weights: the
            # ring is otherwise drained by the time attention needs exp
            warm = res.tile([1, 4], F32)
            nc.vector.memset(warm, 0.0)
            nc.scalar.activation(warm, warm, mybir.ActivationFunctionType.Exp)
            nc.scalar.dma_start(
                out=wo_s, in_=woT_ext.rearrange("(kt p) m -> p kt m", p=128)
            )
            ones_s = res.tile([1, 128], F16)
            nc.vector.memset(ones_s, 1.0)

            # ---- Q/K projections (both dt halves), streamed vs DMA --------
            QT_s = res.tile([128, NDT, SQ], F16)
            KT_s = res.tile([128, NDT, SKV], F16)
            # V with a fused ones column per head: [kv, head, 64+1]
            V_s = res.tile([128, NKT, H_PER, DH + 1], F16)
            nc.vector.memset(V_s[:, :, :, DH : DH + 1], 1.0)

            psp_cm = tc.tile_pool(name="psp", bufs=8, space="PSUM")
            psp = psp_cm.__enter__()
            for which, w_s, dst, src_kt in (
                (0, wq_s, QT_s, qT_kt),
                (1, wk_s, KT_s, kvT_kt),
            ):
                # 8 PSUM banks: 2 dt x 4 qb accumulate in parallel so each
                # input kt tile is consumed once, as it lands
                p_t = {}
                for dt in range(NDT):
                    for qb in range(NQB):
                        p_t[(dt, qb)] = psp.tile(
                            [128, QB], F32, tag="proj", name=f"p{which}{dt}{qb}"
                        )
                for kt in range(NCT):
                    for dt in range(NDT):
                        for qb in range(NQB):
                            nc.tensor.matmul(
                                p_t[(dt, qb)],
                                w_s[:, kt, 128 * dt : 128 * dt + 128],
                                src_kt[kt][:, QB * qb : QB * qb + QB],
                                start=(kt == 0),
                                stop=(kt == NCT - 1),
                            )
                for dt in range(NDT):
                    for qb in range(NQB):
                        # bias-add evacuations alternate Scalar/Vector so the
                        # final bunch drains at 2x and attention starts sooner
                        if (dt * NQB + qb) % 2 == 0:
                            nc.scalar.add(
                                dst[:, dt, QB * qb : QB * qb + QB],
                                p_t[(dt, qb)],
                                bqk_s[:, which * NDT + dt : which * NDT + dt + 1],
                            )
                        else:
                            nc.vector.tensor_scalar_add(
                                dst[:, dt, QB * qb : QB * qb + QB],
                                p_t[(dt, qb)],
                                bqk_s[:, which * NDT + dt : which * NDT + dt + 1],
                            )
            psp_cm.__exit__(None, None, None)

            # ---- V projection: first 4 kv tiles now, rest interleaved -----
            # (pss/pso opened first so psv can close before psy opens: pools
            # must be released in stack order)
            pss_cm = tc.tile_pool(name="pss", bufs=2, space="PSUM")
            pss = pss_cm.__enter__()
            pso_cm = tc.tile_pool(name="pso", bufs=2, space="PSUM")
            pso = pso_cm.__enter__()
            psv_cm = tc.tile_pool(name="psv", bufs=2, space="PSUM")
            psv = psv_cm.__enter__()

            def emit_v(it):
                p_v = psv.tile([128, QB], F32, tag="vproj", name=f"v{it}")
                for kt in range(NCT):
                    nc.tensor.matmul(
                        p_v[:, 0:DG],
                        kvT_kt[kt][:, KT * it : KT * it + KT],
                        wv_s[:, kt, :],
                        start=(kt == 0),
                        stop=False,
                    )
                nc.tensor.matmul(
                    p_v[:, 0:DG], ones_s, bv_s, start=False, stop=True
                )
                nc.vector.tensor_copy(
                    V_s[:, it, :, 0:DH],
                    p_v[:, 0:DG].rearrange("p (h d) -> p h d", h=H_PER),
                )

            for it in range(4):
                emit_v(it)

            # deferred V projections for kv tiles 4..15, drained into PE
            # bubbles while ACT chews on phase-A exps
            vsteps = []
            for it in range(4, NKT):
                box = {}

                def alloc(it=it, box=box):
                    box["t"] = psv.tile(
                        [128, QB], F32, tag="vproj", name=f"v{it}"
                    )

                vsteps.append(alloc)
                for kt in range(NCT):

                    def mm(kt=kt, it=it, box=box):
                        nc.tensor.matmul(
                            box["t"][:, 0:DG],
                            kvT_kt[kt][:, KT * it : KT * it + KT],
                            wv_s[:, kt, :],
                            start=(kt == 0),
                            stop=False,
                        )

                    vsteps.append(mm)

                def fin(it=it, box=box):
                    nc.tensor.matmul(
                        box["t"][:, 0:DG], ones_s, bv_s, start=False, stop=True
                    )
                    nc.vector.tensor_copy(
                        V_s[:, it, :, 0:DH],
                        box["t"][:, 0:DG].rearrange("p (h d) -> p h d", h=H_PER),
                    )

                vsteps.append(fin)
            vi = [0]

            def drain_v(n):
                while n > 0 and vi[0] < len(vsteps):
                    vsteps[vi[0]]()
                    vi[0] += 1
                    n -= 1

            # ---- attention: head pairs (0,1) then (2,3) -------------------
            OT_s = res.tile([128, NDT, SQ], F16)

            def attn_pair(qb, td, interleave):
                n_it = 4 * qb + 4
                o_a = pso.tile([DH + 1, QB], F32, tag="opsum", name="o_a")
                o_b = pso.tile([DH + 1, QB], F32, tag="opsum", name="o_b")
                prev = None
                for it in range(n_it):
                    c_start = max(QB * qb, KT * it)
                    width = QB * (qb + 1) - c_start
                    co = c_start - QB * qb
                    s_pair = pss.tile(
                        [128, 2, QB], F32, tag="spair", name="s_pair"
                    )
                    nc.tensor.matmul(
                        s_pair[:, 0, 0:width],
                        KT_s[0:DH, td, KT * it : KT * it + KT],
                        QT_s[0:DH, td, c_start : c_start + width],
                        start=True,
                        stop=True,
                    )
                    nc.tensor.matmul(
                        s_pair[:, 1, 0:width],
                        KT_s[DH:128, td, KT * it : KT * it + KT],
                        QT_s[DH:128, td, c_start : c_start + width],
                        start=True,
                        stop=True,
                    )
                    p_pair = pp.tile(
                        [128, 2, QB], F16, tag="ptile", name="p_pair"
                    )
                    nc.scalar.activation(
                        p_pair[:, :, 0:width],
                        s_pair[:, :, 0:width],
                        mybir.ActivationFunctionType.Exp,
                    )
                    if it >= 4 * qb:  # diagonal block: causal mask
                        nc.vector.tensor_mul(
                            p_pair[:, :, 0:128],
                            p_pair[:, :, 0:128],
                            mask_s.unsqueeze(1).broadcast_to([128, 2, 128]),
                        )
                    if interleave is not None:
                        interleave(3)
                    # software pipeline: PV for the previous k-tile, so the
                    # PE never waits on the current exp
                    if prev is not None:
                        pp_prev, co_p, w_p, it_p = prev
                        nc.tensor.matmul(
                            o_a[:, co_p : co_p + w_p],
                            V_s[:, it_p, 2 * td, :],
                            pp_prev[:, 0, 0:w_p],
                            start=(it_p == 0),
                            stop=False,
                        )
                        nc.tensor.matmul(
                            o_b[:, co_p : co_p + w_p],
                            V_s[:, it_p, 2 * td + 1, :],
                            pp_prev[:, 1, 0:w_p],
                            start=(it_p == 0),
                            stop=False,
                        )
                    prev = (p_pair, co, width, it)
                pp_prev, co_p, w_p, it_p = prev
                nc.tensor.matmul(
                    o_a[:, co_p : co_p + w_p],
                    V_s[:, it_p, 2 * td, :],
                    pp_prev[:, 0, 0:w_p],
                    start=(it_p == 0),
                    stop=True,
                )
                nc.tensor.matmul(
                    o_b[:, co_p : co_p + w_p],
                    V_s[:, it_p, 2 * td + 1, :],
                    pp_prev[:, 1, 0:w_p],
                    start=(it_p == 0),
                    stop=True,
                )
                return o_a, o_b

            def divnorm(qb, td, o_pair):
                # normalize the two just-finished heads: denominator row
                # PSUM->SBUF f32 (custom-DVE recip misreads PSUM on HW),
                # approx-recip, f16 broadcast from a partition-0 tile
                # (hardware partition_broadcast ignores a nonzero source
                # base), then scale the PV psum directly into OT_s.
                for i_half in range(2):
                    h = 2 * td + i_half
                    d1 = rp.tile([1, QB], F32, tag=f"d{i_half}")
                    nc.vector.tensor_copy(d1, o_pair[i_half][DH : DH + 1, :])
                    r2h = rp.tile([1, QB], F32, tag=f"r2{i_half}")
                    nc.vector.reciprocal_approx_fast(r2h, d1)
                    r16h = rp.tile([1, QB], F16, tag=f"r16{i_half}")
                    nc.vector.tensor_copy(r16h, r2h)
                    rb = rp.tile([DH, QB], F16, tag=f"rb{i_half}")
                    nc.gpsimd.partition_broadcast(rb, r16h, channels=DH)
                    p0 = DH * (h % 2)
                    td2 = h // 2
                    nc.vector.tensor_mul(
                        OT_s[p0 : p0 + DH, td2, QB * qb : QB * qb + QB],
                        o_pair[i_half][0:DH, :],
                        rb,
                    )

            # phase A: head pair (0,1) with V projections interleaved.
            # V_s[:, it] must be fully emitted before any PV matmul that
            # reads it: guarantee coverage of it < 4*qb+4 at each loop head.
            for qb in range(NQB):
                drain_v(max(0, 40 * qb - vi[0]))
                o_pair = attn_pair(qb, 0, drain_v)
                drain_v(6)
                divnorm(qb, 0, o_pair)
            drain_v(len(vsteps))
            psv_cm.__exit__(None, None, None)

            psy_cm = tc.tile_pool(name="psy", bufs=2, space="PSUM")
            psy = psy_cm.__enter__()

            ysteps = []
            yi = [0]

            def push_y(qb):
                # out-projection for q-block qb as deferred steps, drained
                # into PE bubbles of the next attention pair
                for mt in range(D // 128):
                    box = {}

                    def alloc(box=box):
                        box["t"] = psy.tile(
                            [128, QB], F32, tag="ypsum", name="y_q"
                        )

                    ysteps.append(alloc)
                    for kt2 in range(NDT):

                        def mm(kt2=kt2, mt=mt, qb=qb, box=box):
                            nc.tensor.matmul(
                                box["t"],
                                wo_s[:, kt2, 128 * mt : 128 * mt + 128],
                                OT_s[:, kt2, QB * qb : QB * qb + QB],
                                start=(kt2 == 0),
                                stop=(kt2 == NDT - 1),
                            )

                        ysteps.append(mm)

                    def fin(mt=mt, qb=qb, box=box):
                        y_sb = yp.tile([128, QB], F16, tag="ysb", name="y_sb")
                        if mt % 4 == 3:
                            nc.scalar.copy(y_sb, box["t"])
                        else:
                            nc.vector.tensor_copy(y_sb, box["t"])
                        nc.sync.dma_start(
                            out=yT_ext[
                                128 * mt : 128 * mt + 128,
                                QB * qb : QB * qb + QB,
                            ],
                            in_=y_sb,
                        )

                    ysteps.append(fin)

            def drain_y(n, keep=0):
                while n > 0 and yi[0] < len(ysteps) - keep:
                    ysteps[yi[0]]()
                    yi[0] += 1
                    n -= 1

            # phase B: head pair (2,3); out-proj steps trail one q-block
            # behind attention, drained into PE bubbles. During the last
            # attention pair, hold back 16 y-steps of the previous q-block:
            # released after the final divnorm is emitted, they give the PE
            # independent work while that serial divide chain runs.
            def drain_y_hold(n):
                drain_y(n, keep=16)

            for qb in range(NQB):
                fn = drain_y_hold if qb == NQB - 1 else drain_y
                o_pair = attn_pair(qb, 1, fn)
                divnorm(qb, 1, o_pair)
                push_y(qb)
            drain_y(len(ysteps))
            if dbg:
                nc.sync.dma_start(
                    out=dbg["v"][:, :],
                    in_=V_s.rearrange("p a b c -> p (a b c)"),
                )
                nc.sync.dma_start(
                    out=dbg["qt"][:, :], in_=QT_s.rearrange("p a b -> p (a b)")
                )
                nc.sync.dma_start(
                    out=dbg["kt"][:, :], in_=KT_s.rearrange("p a b -> p (a b)")
                )
                nc.sync.dma_start(
                    out=dbg["ot"][:, :], in_=OT_s.rearrange("p a b -> p (a b)")
                )
            psy_cm.__exit__(None, None, None)
            pso_cm.__exit__(None, None, None)
            pss_cm.__exit__(None, None, None)

    nc.finalize()
    return nc


def _get_nc():
    if "nc" not in _CACHE:
        _CACHE["nc"] = _build()
    return _CACHE["nc"]


def _prep_core_inputs(c, query, key_value, Wq, bq, Wk, bk, Wv, bv, Wo, bo):
    b = c // HG
    hg = c % HG
    hs = slice(DG * hg, DG * hg + DG)
    scale = 1.0 / np.sqrt(DH)

    bqk = np.zeros((128, 2 * NDT), np.float32)
    bq_s = (bq[hs] * scale).astype(np.float32)
    bk_s = bk[hs].astype(np.float32)
    for dt in range(NDT):
        bqk[:, dt] = bq_s[128 * dt : 128 * dt + 128]
        bqk[:, NDT + dt] = bk_s[128 * dt : 128 * dt + 128]

    kk, qq = np.meshgrid(np.arange(128), np.arange(128), indexing="ij")
    mask = (qq >= kk).astype(np.float16)

    return {
        "qT": np.ascontiguousarray(query[b].T).astype(np.float16),
        "kvT": np.ascontiguousarray(key_value[b].T).astype(np.float16),
        "wqT": np.ascontiguousarray((Wq[hs, :] * scale).T).astype(np.float16),
        "wkT": np.ascontiguousarray(Wk[hs, :].T).astype(np.float16),
        "wvT": np.ascontiguousarray(Wv[hs, :].T).astype(np.float16),
        "woT": np.ascontiguousarray(Wo[:, hs].T).astype(np.float16),
        "bqk": bqk,
        "bv": bv[hs].reshape(1, DG).astype(np.float16),
        "mask": mask,
    }


def kernel(
    query,
    key_value,
    Wq,
    bq,
    Wk,
    bk,
    Wv,
    bv,
    Wo,
    bo,
    _trace=False,
):
    query = np.asarray(query)
    key_value = np.asarray(key_value)
    args = [np.asarray(a) for a in (Wq, bq, Wk, bk, Wv, bv, Wo, bo)]

    nc = _get_nc()
    in_maps = [
        _prep_core_inputs(c, query, key_value, *args) for c in range(N_CORES)
    ]
    res = run_bass_kernel_spmd(
        nc, in_maps, list(range(N_CORES)), trace=_trace
    )

    out = np.zeros((B, SQ, D), np.float32)
    for c in range(N_CORES):
        out[c // HG] += res.results[c]["yT"].astype(np.float32).T
    out += args[7].astype(np.float32)  # bo
    if _trace:
        return out, res
    return out
